# revision 43
# baseline (speedup 1.0000x reference)
"""Trainium2 Bass kernel for nn_Head_72507637891886.

Computes r = exp(-(|k|_F^2+|q|_F^2)/2) * mean(cosh((k+q) @ w), -1) where
k = x@wk+bk, q = x@wq+bq, w = sqrt(32) * w_raw.T / |w_raw|_F.

Strategy: data-parallel over batch (2 batches = 8192 tokens per core, 8
cores). The kernel is HBM-bound on streaming x (modeled 360 GB/s, fully
serialized across DMA queues), so x is shipped to the device as fp8-e4m3
([E, TOK] transposed on host): quarters the f32 stream to 23.3us. Each
512-token block's rows are exactly 512B, dodging the sub-512B descriptor
penalty; the wall8/bias constants stream behind x0/x1, one per gap, so
their HWDGE setup stages never bubble the x stream.

The matmul runs in fp8 DoubleRow perf mode (two K=128 chunks contracted
per instruction at 0.5 cycles/row; dual-fp8 LdWeights requires the full
128 stationary columns), and the cosh projection is folded into the same
matmul: y = (k+q)@wS = x@((wk+wq)@wS) + const, and PE cost only scales
with moving columns. Stationary layout [16*wY8 | 56 zero pad | 64*wkq]
puts y8^T = [y;-y] on PSUM rows 0:8 (gpsimd needs partition base 0 - the
all-reduce reads the wrong partitions from base 64) and kq^T on rows
64:128 (ACT/DVE need a 32-aligned base); the x64/x16 scales sit the
weights in e4m3's normal range and are undone by ACT scale / host /4096.

Engine assignment is strictly feed-forward (PE -> ACT -> Pool/DVE). Any
PE instruction that waits on an ACT result (e.g. a mean-reduce matmul
over the exponentials) blocks the in-order PE sequencer between matmul
quads and stretches the pipeline period past the 1.46us DMA period, so
the feature-dim mean runs on the otherwise-idle GPSIMD instead. Per
512-token block (engine busy / 1456ns period):
  - PE:   4 DoubleRow matmuls -> [16*y8; pad; 64*kq] PSUM      (534ns)
  - ACT:  e' = Exp(y8/16 + bY8 + ln(1/8)) bf16; kqb = kq+64b   (1224ns)
  - Pool: partition_all_reduce(e') -> r row of [8, TOK] SBUF    (806ns)
  - DVE:  (kq+64b)*kqb via scalar_tensor_tensor, accum -> ss col (658ns)
ACT's Exp/Square tables preload via dummy activations at t=0 so the
1.28us LoadActFuncSet hides behind the fill; dummy matmuls warm the PE
p-state ramp.

Drain (the last byte's semaphore alone costs 900ns): block 14's kqb goes
to the idle-by-then DVE so ACT's chain is Exp14, Exp15, Sq15; block 15's
squares use ACT Square+accum_out after Exp15 (a DVE path would queue 5
deep); rout leaves on the SP HWDGE right after the final all-reduce while
ss generates on the Pool SWDGE in parallel (HWDGE is device-exclusive).
Host gathers r, all-reduces the sum-of-squares partials (/4096), and
applies the exp(-z2/2) scale (underflows to 0 for this input scale).
"""

import numpy as np

B, T, E, D = 16, 4096, 1024, 32
OMEGA = 4
NCORES = 8
TOK = B * T // NCORES  # 8192 tokens per core
KC = E // 128          # 8 contraction chunks (4 DoubleRow pairs)
BLK = 512
NBLK = TOK // BLK      # 16 blocks
NW = 128               # 64 kq | 8 y8 | 56 zero pad (dual-fp8 LdWeights needs 128)

_CACHE = {}
LAST_RESULTS = None  # BassKernelResults from the most recent run (for test.py)
LAST_PROFILE = None
LAST_OUTS = None
TRACE = False


def _build_bass():
    import concourse.mybir as mybir
    import concourse.tile as tile
    from concourse import bacc, bass_isa

    f32 = mybir.dt.float32
    bf16 = mybir.dt.bfloat16
    f8 = mybir.dt.float8e4
    AF = mybir.ActivationFunctionType
    ALU = mybir.AluOpType
    DR = mybir.MatmulPerfMode.DoubleRow
    RADD = bass_isa.ReduceOp.add

    nc = bacc.Bacc()
    xt = nc.declare_dram_parameter("xt", [E, TOK], f8, isOutput=False)
    wall8 = nc.declare_dram_parameter("wall8", [128, KC * NW], f8, isOutput=False)
    bias2 = nc.declare_dram_parameter("bias2", [128, 2], f32, isOutput=False)
    rout = nc.declare_dram_parameter("rout", [1, TOK], f32, isOutput=True)
    # ss[f, i] = sum over block i's tokens of (64*(kq+b))^2; host /4096
    ss = nc.declare_dram_parameter("ss", [2 * D, NBLK], f32, isOutput=True)

    with tile.TileContext(nc) as tc:
        with (
            tc.tile_pool(name="const", bufs=1) as const,
            tc.tile_pool(name="xp", bufs=5) as xp,
            tc.tile_pool(name="work", bufs=3) as work,
            tc.tile_pool(name="acc", bufs=1) as acc,
            tc.tile_pool(name="kqps", bufs=3, space="PSUM") as kqps,
        ):
            # ACT table preload: dummy Exp on a junk tile so the 1.28us
            # LoadActFuncSet runs behind the fill, not before Exp(0).
            wu = const.tile([128, 512], bf16)
            nc.vector.memset(wu, 0.0)
            junk = const.tile([1, 2], bf16)
            nc.scalar.activation(junk[:, 0:1], wu[0:1, 0:1], AF.Exp)
            nc.scalar.activation(junk[:, 1:2], wu[0:1, 0:1], AF.Square)
            # PE p-state warmup behind the fill (0.65 -> 2.4 GHz ramp)
            wu_ps = kqps.tile([64, 512], f32, tag="kq", name="wu_ps")
            for _ in range(6):
                nc.tensor.matmul(wu_ps, wu[:, 0:64], wu, start=True, stop=True)

            wall8_sb = const.tile([128, KC, NW], f8)
            bias2_sb = const.tile([128, 2], f32)
            bY8_sb = bias2_sb[0:8, 1:2]        # [bY; -bY] + ln(1/8)
            bkq_sb = bias2_sb[64:128, 0:1]     # 64 * [bk|bq]

            # y8 sits on PSUM rows 0:8 and kq on rows 8:72 so the gpsimd
            # all-reduce operates at partition base 0 (nonzero bases read
            # the wrong partitions on hardware)
            ss_sb = acc.tile([128, NBLK], f32)
            r8_sb = acc.tile([8, TOK], f32)

            kq_t = [None] * NBLK   # [64*kq; 16*y8] PSUM tiles
            e_t = [None] * NBLK    # e' = [e^y; e^-y]/8 bf16 (rows 64:72)

            def stage_exp(i, lo=0, hi=BLK):
                if e_t[i] is None:
                    e_t[i] = work.tile([8, BLK], bf16, tag="e", name="e")
                nc.scalar.activation(
                    e_t[i][0:8, lo:hi], kq_t[i][0:8, lo:hi], AF.Exp,
                    bias=bY8_sb, scale=1.0 / 16.0,
                )

            def stage_red(i, lo=0, hi=BLK):
                nc.gpsimd.partition_all_reduce(
                    r8_sb[:, i * BLK + lo : i * BLK + hi],
                    e_t[i][0:8, lo:hi],
                    channels=8,
                    reduce_op=RADD,
                )

            def stage_ss(i, last=False):
                # kqb = kq + 64b on ACT (it has slack; DVE at 90% was the
                # drain tail), then one DVE scalar_tensor_tensor squares it
                # against the PSUM kq with exact bias and accumulates the
                # ss column. The last block does both on DVE (ACT is busy
                # with the final Exps then).
                kqb = work.tile([128, BLK], bf16, tag="kqb", name="kqb")
                if last:
                    nc.vector.tensor_scalar_add(kqb[64:128, :], kq_t[i][64:128, :],
                                                bkq_sb)
                else:
                    nc.scalar.activation(kqb[64:128, :], kq_t[i][64:128, :],
                                         AF.Identity, bias=bkq_sb)
                sq = work.tile([128, BLK], bf16, tag="sq", name="sq")
                nc.vector.scalar_tensor_tensor(
                    out=sq[64:128, :],
                    in0=kq_t[i][64:128, :],
                    scalar=bkq_sb,
                    in1=kqb[64:128, :],
                    op0=ALU.add,
                    op1=ALU.mult,
                    accum_out=ss_sb[64:128, i : i + 1],
                )

            for ib in range(NBLK):
                t0 = ib * BLK
                x_tile = xp.tile([128, KC, BLK], f8, tag="x")
                if ib == NBLK - 1:
                    # split the final transfer by contraction pairs: the first
                    # pairs' matmuls start a transfer earlier, shortening the
                    # post-stream drain (same bytes, both runs >= 512B rows)
                    nc.sync.dma_start(
                        out=x_tile[:, 0:6, :],
                        in_=xt[0 : 6 * 128, t0 : t0 + BLK].rearrange(
                            "(c p) t -> p c t", p=128
                        ),
                    )
                    nc.sync.dma_start(
                        out=x_tile[:, 6:KC, :],
                        in_=xt[6 * 128 : E, t0 : t0 + BLK].rearrange(
                            "(c p) t -> p c t", p=128
                        ),
                    )
                else:
                    nc.sync.dma_start(
                        out=x_tile,
                        in_=xt[:, t0 : t0 + BLK].rearrange("(c p) t -> p c t", p=128),
                    )
                if ib == 0:
                    # small operands load behind block 0's transfer; emitted
                    # BEFORE their first readers so the tile framework sees
                    # the true write->read dependency
                    nc.sync.dma_start(
                        out=wall8_sb,
                        in_=wall8[:].rearrange("p (c m) -> p c m", c=KC),
                    )
                elif ib == 1:
                    # behind x1: four HWDGE setups before x1 would gap the
                    # stream by ~100ns
                    nc.sync.dma_start(out=bias2_sb, in_=bias2[:])

                kq_t[ib] = kqps.tile([NW, BLK], f32, tag="kq", name="kq")
                for j in range(KC // 2):
                    nc.tensor.matmul(
                        kq_t[ib],
                        wall8_sb[:, 2 * j : 2 * j + 2, :],
                        x_tile[:, 2 * j : 2 * j + 2, :],
                        start=(j == 0),
                        stop=(j == KC // 2 - 1),
                        perf_mode=DR,
                    )

                if ib >= 1:
                    stage_exp(ib - 1)
                    stage_red(ib - 1)
                    # block 14's kqb goes to the (idle-by-then) DVE so the
                    # ACT drain chain is purely Exp14,Exp15a,Exp15b,Sq15
                    stage_ss(ib - 1, last=(ib - 1 == NBLK - 2))

            # Drain: half-granular Exp/all-reduce so rout_b leaves early;
            # ss for block 15 on DVE in parallel.
            L = NBLK - 1
            stage_exp(L)
            stage_red(L)
            # block 15's squares go on ACT after the final Exps: the DVE
            # path would serialize 5 ops deep into the drain, while ACT
            # finishes Sq15+accum at the same time rout's chain completes
            sq15 = work.tile([128, BLK], bf16, tag="sq", name="sq15")
            nc.scalar.activation(
                sq15[64:128, :], kq_t[L][64:128, :], AF.Square, bias=bkq_sb,
                accum_out=ss_sb[64:128, L : L + 1],
            )
            nc.sync.dma_start(out=rout[:], in_=r8_sb[0:1, :])
            nc.gpsimd.dma_start(out=ss[:], in_=ss_sb[64:128, :])
    nc.compile()
    return nc


def _get_nc():
    if "nc" not in _CACHE:
        _CACHE["nc"] = _build_bass()
    return _CACHE["nc"]


def _make_inputs(x, wq, bq, wk, bk, w_raw):
    import ml_dtypes

    e4m3 = ml_dtypes.float8_e4m3
    # replicated small operands: stationary wall [64*wkq | 16*wY8 | 0] per chunk
    wkq = np.concatenate([wk, wq], axis=1)  # [E, 64]
    wt = w_raw.T.astype(np.float32)  # [D, OMEGA]
    norm = np.sqrt(np.sum(wt ** 2, dtype=np.float32))
    wS = (np.float32(np.sqrt(np.float32(D))) * (wt / norm)).astype(np.float32)
    wY = (wk + wq) @ wS                      # [E, OMEGA]
    wY8 = np.concatenate([wY, -wY], axis=1)  # [E, 8]
    big = np.concatenate(
        [wY8 * 16.0, np.zeros((E, NW - 72), np.float32), wkq * 64.0], axis=1
    )  # [E, 128]: 8 y8 | 56 zero pad | 64 kq (bases 0 and 64 keep the
    # gpsimd all-reduce and the ACT/DVE kq ops partition-aligned)
    wall8 = np.ascontiguousarray(
        big.reshape(KC, 128, NW).transpose(1, 0, 2).reshape(128, KC * NW)
    ).astype(e4m3)

    bY = (bk + bq) @ wS                      # [OMEGA]
    bias2 = np.zeros((128, 2), dtype=np.float32)
    bias2[0:8, 1] = np.concatenate([bY, -bY]) + np.float32(np.log(0.125))
    bias2[64:128, 0] = 64.0 * np.concatenate([bk, bq])

    in_maps = []
    bpc = B // NCORES
    for c in range(NCORES):
        xt = np.ascontiguousarray(
            x[c * bpc : (c + 1) * bpc].reshape(TOK, E).astype(e4m3).T
        )  # [E, TOK] fp8
        in_maps.append({"xt": xt, "wall8": wall8, "bias2": bias2})
    return in_maps


def kernel(x, wq, bq, wk, bk, wv, bv, w_raw):
    global LAST_RESULTS, LAST_OUTS
    from concourse.bass_utils import run_bass_kernel_spmd

    x = np.asarray(x, dtype=np.float32)
    wq = np.asarray(wq, dtype=np.float32)
    bq = np.asarray(bq, dtype=np.float32)
    wk = np.asarray(wk, dtype=np.float32)
    bk = np.asarray(bk, dtype=np.float32)
    w_raw = np.asarray(w_raw, dtype=np.float32)

    in_maps = _make_inputs(x, wq, bq, wk, bk, w_raw)

    nc = _get_nc()
    res = run_bass_kernel_spmd(
        nc, in_maps, core_ids=list(range(NCORES)), trace=False
    )
    LAST_RESULTS = res
    results = res.results
    LAST_OUTS = results

    r_parts = []
    ss = 0.0
    for out in results:
        r_parts.append(out["rout"].reshape(TOK))
        ss += float(out["ss"].sum(dtype=np.float64)) / 4096.0

    with np.errstate(under="ignore"):
        a = np.float32(np.exp(np.float64(-ss / 2.0)))
    r = (a * np.concatenate(r_parts)).reshape(B, T).astype(np.float32)
    return r
